# revision 41
# baseline (speedup 1.0000x reference)
"""Multi-head attention TRN2 kernel (B=4, S=2048, E=128, H=8) on 8 NeuronCores.

Sharding: core c handles batch b = c // 2 and head group g = c % 2
(heads 4g .. 4g+3).  Each core computes the partial output
outT_partial[e_out, s] = sum_{h in group} (softmax(QK^T/sqrt(E)) V)_h @ Wo_h
for its batch, transposed.  Host sums the two head-group partials per batch,
transposes, and adds bo (plus the host-folded bv and bk contributions).

v2 design (vs v1 baseline at ~204 us):
  - host sends q pre-transposed as f16 [E, S]: no PE transposes, no qT casts
  - bk dropped entirely (adds a per-query constant to scores -> cancels in
    softmax); bv folded into the host-side output bias (as before)
  - attn weights (exp of scores) written by ScalarE directly as fp8e4;
    V projection cast to fp8e4 -> AV matmul and the softmax-denominator
    matmul both run in DoubleRow mode (contract 2 t-blocks of 128 per
    matmul), halving PE time for those stages
  - denominator computed as a tail burst of 8 DoubleRow matmuls with an
    fp8 ones matrix (no DVE fold tree at all)
  - PSUM: sc [128,1024] x2bufs (4 banks) + zts 2 banks + 2 rotating work
    banks (proj / V / dns / outproj)
  - next head's Q/K projections and the V projection are drip-fed into the
    tb loop (2 ops per iteration) so PE work overlaps the exp stream
"""

import sys

for _p in ("/opt/trn_rl_repo",):
    if _p not in sys.path:
        sys.path.insert(0, _p)

import numpy as np

import concourse.bass as bass
import concourse.mybir as mybir
import concourse.tile as tile
from concourse.bass_utils import run_bass_kernel_spmd

F32 = mybir.dt.float32
F16 = mybir.dt.float16
F8 = mybir.dt.float8e4
DR = mybir.MatmulPerfMode.DoubleRow
EXP = mybir.ActivationFunctionType.Exp

B, S, E, H = 4, 2048, 128, 8
NH = 4          # heads per core
TB = S // 128   # 16 t blocks
SW = 1024       # s-half width
NC = 512        # psum-bank chunk
SCALE = 1.0 / np.sqrt(E)

_prog_cache = {}


def build_program():
    if "nc" in _prog_cache:
        return _prog_cache["nc"]

    import concourse.bacc as bacc

    nc = bacc.Bacc("TRN2", target_bir_lowering=False, debug=False)

    qt_d = nc.dram_tensor("qT", [E, S], F16, kind="ExternalInput").ap()
    # packed weights: dim1 = (Wq, Wk, Wv, Wo); 4KB DRAM rows for DMA speed
    w_d = nc.dram_tensor("W", [E, 4, NH, E], F16, kind="ExternalInput").ap()
    bq_d = nc.dram_tensor("bq", [E, NH], F32, kind="ExternalInput").ap()
    out_d = nc.dram_tensor("out", [E, S], F32, kind="ExternalOutput").ap()

    with tile.TileContext(nc) as tc:
        _emit(nc, tc, qt_d, w_d, bq_d, out_d)

    nc.compile()
    _prog_cache["nc"] = nc
    return nc


def _emit(nc, tc, qt_d, w_d, bq_d, out_d):
    from collections import deque
    from contextlib import ExitStack

    ctx = ExitStack()
    consts = ctx.enter_context(tc.tile_pool(name="consts", bufs=1))
    heads = ctx.enter_context(tc.tile_pool(name="heads", bufs=2))
    attns = ctx.enter_context(tc.tile_pool(name="attns", bufs=2))
    works = ctx.enter_context(tc.tile_pool(name="works", bufs=2))
    psum_sc = ctx.enter_context(tc.tile_pool(name="psum_sc", bufs=2, space="PSUM"))
    psum_av = ctx.enter_context(tc.tile_pool(name="psum_av", bufs=2, space="PSUM"))
    psum_wk = ctx.enter_context(tc.tile_pool(name="psum_wk", bufs=2, space="PSUM"))

    # ---- constants / inputs ----
    # DMA issue cost is ~650ns per call per engine queue and per-queue
    # bandwidth is poor for small rows; use few, large-row transfers split
    # across the two HWDGE issue queues (Sync + ScalarE), ordered by use.
    qT = consts.tile([128, S], F16, tag="qT")  # [e, s]
    # partition-split the q transfer across two queues (4KB rows each half)
    nc.sync.dma_start(out=qT[0:64, :], in_=qt_d[0:64, :])
    nc.gpsimd.dma_start(out=qT[64:128, :], in_=qt_d[64:128, :])
    w_all4 = consts.tile([128, 4, NH, 128], F16, tag="w_all4")
    nc.scalar.dma_start(out=w_all4[:, 0:2], in_=w_d[:, 0:2])  # Wq+Wk first
    bq = consts.tile([128, NH], F32, tag="bq")  # [f, h]
    nc.scalar.dma_start(out=bq, in_=bq_d)
    nc.scalar.dma_start(out=w_all4[:, 2:4], in_=w_d[:, 2:4])
    wq = w_all4[:, 0]  # [e_in, h, e_out]
    wk = w_all4[:, 1]
    wv = w_all4[:, 2]
    wo = w_all4[:, 3]  # [f, h, g]

    ones8 = consts.tile([128, 2, 128], F8, tag="ones8")
    nc.vector.memset(ones8, 1.0)

    v8 = consts.tile([128, TB, NH * 128], F8, tag="v8")  # [t_sub, tb, (h f)]
    wv_all = wv.rearrange("e h f -> e (h f)")

    acc_a = [
        consts.tile([128, SW], F32, tag=f"acca{s}", name=f"acca{s}") for s in range(2)
    ]
    acc_b = [
        consts.tile([128, SW], F32, tag=f"accb{s}", name=f"accb{s}") for s in range(2)
    ]

    def v_block(tb):
        ps = psum_wk.tile([128, NC], F32, tag="work", name=f"vps_{tb}")
        nc.tensor.matmul(ps, lhsT=qT[:, tb * 128 : (tb + 1) * 128], rhs=wv_all,
                         start=True, stop=True)
        nc.vector.tensor_copy(v8[:, tb, :], ps)

    def proj_q(h, qt_h, j):
        ps = psum_wk.tile([128, NC], F32, tag="work", name=f"qp{h}_{j}")
        nc.tensor.matmul(ps, lhsT=wq[:, h, :], rhs=qT[:, j * 512 : (j + 1) * 512],
                         start=True, stop=True)
        nc.vector.tensor_scalar_add(qt_h[:, j * 512 : (j + 1) * 512], ps,
                                    bq[:, h : h + 1])

    def proj_k(h, kt_h, j):
        ps = psum_wk.tile([128, NC], F32, tag="work", name=f"kp{h}_{j}")
        nc.tensor.matmul(ps, lhsT=wk[:, h, :], rhs=qT[:, j * 512 : (j + 1) * 512],
                         start=True, stop=True)
        nc.vector.tensor_copy(kt_h[:, j * 512 : (j + 1) * 512], ps)

    def alloc_head(h):
        qt_h = heads.tile([128, S], F16, tag="QT", name=f"qt{h}")  # [f, s]
        kt_h = heads.tile([128, S], F16, tag="KT", name=f"kt{h}")  # [f, t]
        return qt_h, kt_h

    # prologue: only what the first scores matmul needs, evacuated on the
    # (otherwise idle) ScalarE so the DVE isn't on the critical path
    cur = alloc_head(0)
    ps = psum_wk.tile([128, NC], F32, tag="work", name="qp0_0p")
    nc.tensor.matmul(ps, lhsT=wq[:, 0, :], rhs=qT[:, 0:512], start=True, stop=True)
    nc.scalar.add(cur[0][:, 0:512], ps, bq[:, 0:1])
    ps = psum_wk.tile([128, NC], F32, tag="work", name="kp0_0p")
    nc.tensor.matmul(ps, lhsT=wk[:, 0, :], rhs=qT[:, 0:512], start=True, stop=True)
    nc.scalar.copy(cur[1][:, 0:512], ps)
    ps = psum_wk.tile([128, NC], F32, tag="work", name="qp0_1p")
    nc.tensor.matmul(ps, lhsT=wq[:, 0, :], rhs=qT[:, 512:1024], start=True, stop=True)
    nc.scalar.add(cur[0][:, 512:1024], ps, bq[:, 0:1])

    osb_tiles = {}

    def out_proj(h, sh, c, ztn):
        # output projection + head accumulation for chunk c of segment (h, sh)
        wo_ps = psum_wk.tile([128, NC], F32, tag="work", name=f"wop{h}{sh}{c}")
        nc.tensor.matmul(wo_ps, lhsT=wo[:, h, :], rhs=ztn, start=True, stop=True)
        asl = slice(c * 512, (c + 1) * 512)
        if h == 0:
            nc.vector.tensor_copy(acc_a[sh][:, asl], wo_ps)
        elif h == 1:
            nc.vector.tensor_add(acc_b[sh][:, asl], acc_a[sh][:, asl], wo_ps)
        elif h == 2:
            nc.vector.tensor_add(acc_a[sh][:, asl], acc_b[sh][:, asl], wo_ps)
        else:
            if sh not in osb_tiles:
                osb_tiles[sh] = works.tile([128, SW], F32, tag="osb",
                                           name=f"osb{sh}")
            osb = osb_tiles[sh]
            nc.vector.tensor_add(osb[:, asl], acc_a[sh][:, asl], wo_ps)
            # ship each chunk as soon as it's ready, partition-split across
            # two issue queues
            csl = slice(sh * SW + c * 512, sh * SW + (c + 1) * 512)
            nc.sync.dma_start(out=out_d[0:64, csl], in_=osb[0:64, asl])
            nc.scalar.dma_start(out=out_d[64:128, csl], in_=osb[64:128, asl])

    nxt = None
    deferred = None  # tail work from the previous segment
    for h in range(NH):
        qt_h, kt_h = cur
        for sh in range(2):
            # per-iteration emission schedule. pre[tb] runs at the top of
            # iteration tb (projection drip / deferred out-proj: deps are
            # already met, so they never head-block the FIFO); post[tb] runs
            # after scores+exp of iteration tb (AV/dns pairs whose exp dep
            # clears exactly when the PE FIFO reaches them).
            pre = [[] for _ in range(TB + 1)]
            post = [[] for _ in range(TB + 1)]

            def place(tb, thunk):
                pre[min(tb, TB)].append(thunk)

            def place_post(tb, thunk):
                post[min(tb, TB)].append(thunk)

            # tail of the previous segment: its last matmuls go after this
            # segment's sc(1) (same exp gate — sc(1) must issue first); then
            # per-chunk chains norm0 / wo0 / norm1 / wo1 so chunk 0's work
            # bank recycles as early as possible.
            # previous segment's tail, then its normalization, then its
            # out-projection, then this segment's projection drip, then the
            # denominator accumulation — strictly in that order so the two
            # rotating work banks never force a FIFO head-block.
            base = 0
            if deferred is not None:
                n_mms = len(deferred["mms"])
                for i, t in enumerate(deferred["mms"]):
                    place_post(1 + i // 2, t)
                base = 1 + (n_mms + 1) // 2
                place_post(base, deferred["norm"][0])
                place_post(base + 1, deferred["wo"][0])
                place_post(base + 1, deferred["norm"][1])
                place_post(base + 2, deferred["wo"][1])
                deferred = None

            # sh1 segments carry the next head's projections (1 op/slot) and
            # defer most of their denominator work into the drip-free sh0
            # segments, which have the PE slack to absorb it.
            drip = deque()
            drip0 = 0
            per_slot = 2
            if h == 0 and sh == 0:
                vb = lambda t: (lambda tt=t: v_block(tt))
                pk = lambda j: (lambda jj=j: proj_k(0, kt_h, jj))
                pq = lambda j: (lambda jj=j: proj_q(0, qt_h, jj))
                drip += [vb(0), vb(1), pk(1), vb(2), vb(3), pk(2), vb(4), vb(5),
                         pq(2), vb(6), vb(7), pk(3), vb(8), vb(9), pq(3)]
                drip += [vb(t) for t in range(10, TB)]
            if sh == 1 and h + 1 < NH:
                nxt = alloc_head(h + 1)
                hh, nq, nk = h + 1, nxt[0], nxt[1]
                drip += [lambda j=j: proj_q(hh, nq, j) for j in range(4)]
                drip += [lambda j=j: proj_k(hh, nk, j) for j in range(4)]
                drip0, per_slot = 4, 1
            n_drip = len(drip)
            for i, t in enumerate(drip):
                place_post(drip0 + i // per_slot, t)
            first_free = max(drip0 + (n_drip + per_slot - 1) // per_slot + 1,
                             5, base + 3)

            s0 = sh * SW
            attnT = attns.tile([128, TB, SW], F8, tag="attnT", name=f"at{h}{sh}")
            zts = [
                psum_av.tile([128, NC], F32, tag="zt", name=f"zt{h}{sh}{c}")
                for c in range(2)
            ]
            dn_tiles = {}

            def dns_pair(p, c, at=attnT, dn=dn_tiles, hh=h, ss=sh):
                if c not in dn:
                    dn[c] = psum_wk.tile(
                        [128, NC], F32, tag="work", name=f"dn{hh}{ss}{c}"
                    )
                nc.tensor.matmul(
                    dn[c],
                    lhsT=ones8,
                    rhs=at[:, 2 * p : 2 * p + 2, c * 512 : (c + 1) * 512],
                    start=(p == 0), stop=(p == 7), perf_mode=DR,
                )

            def av_pair(p, c, at=attnT, z=zts, hh=h):
                nc.tensor.matmul(
                    z[c],
                    lhsT=v8[:, 2 * p : 2 * p + 2, hh * 128 : (hh + 1) * 128],
                    rhs=at[:, 2 * p : 2 * p + 2, c * 512 : (c + 1) * 512],
                    start=(p == 0), stop=(p == 7), perf_mode=DR,
                )

            for p in range(8):
                place_post(
                    max(2 * p + 3, first_free + p),
                    lambda p=p, f=dns_pair: (f(p, 0), f(p, 1)),
                )
                place_post(
                    2 * p + 3, lambda p=p, f=av_pair: (f(p, 0), f(p, 1))
                )

            for tb in range(TB):
                for t in pre[tb]:
                    t()
                sc = psum_sc.tile([128, SW], F32, tag="sc", name=f"sc{h}{sh}{tb}")
                for c in range(2):
                    nc.tensor.matmul(
                        sc[:, c * 512 : (c + 1) * 512],
                        lhsT=kt_h[:, tb * 128 : (tb + 1) * 128],
                        rhs=qt_h[:, s0 + c * 512 : s0 + (c + 1) * 512],
                        start=True, stop=True,
                    )
                nc.scalar.activation(attnT[:, tb, :], sc, EXP, scale=SCALE)
                for t in post[tb]:
                    t()

            holder = {}

            def norm_chunk(c, dn=dn_tiles, z=zts, hol=holder, hh=h, ss=sh):
                recip = works.tile([128, NC], F32, tag="recip",
                                   name=f"rc{hh}{ss}{c}")
                nc.vector.reciprocal_approx_fast(recip, dn[c])
                ztn = works.tile([128, NC], F16, tag="ztn", name=f"zn{hh}{ss}{c}")
                nc.vector.tensor_mul(ztn, z[c], recip)
                hol[c] = ztn

            if h == NH - 1 and sh == 1:
                # final segment: drain immediately, complete per-chunk chains
                for t in pre[TB]:
                    t()
                for t in post[TB]:
                    t()
                for c in range(2):
                    norm_chunk(c)
                    out_proj(h, sh, c, holder[c])
            else:
                deferred = {
                    "mms": pre[TB] + post[TB],
                    "norm": [lambda c=c, f=norm_chunk: f(c) for c in range(2)],
                    "wo": [
                        lambda c=c, hol=holder, hh=h, ss=sh: out_proj(
                            hh, ss, c, hol[c]
                        )
                        for c in range(2)
                    ],
                }
        if h + 1 < NH:
            cur = nxt

    ctx.close()


def _in_maps(inputs):
    q = np.asarray(inputs["q"], dtype=np.float32)
    Wq = np.asarray(inputs["Wq"], dtype=np.float32)
    bq = np.asarray(inputs["bq"], dtype=np.float32)
    Wk = np.asarray(inputs["Wk"], dtype=np.float32)
    Wv = np.asarray(inputs["Wv"], dtype=np.float32)
    Wo = np.asarray(inputs["Wo"], dtype=np.float32).reshape(H, E, E)

    def warr(w, hs):  # [h, e_in, e_out] slice -> [e_in, h, e_out] f16
        return w[hs].transpose(1, 0, 2).astype(np.float16)

    maps = []
    for c in range(8):
        b = c // 2
        hs = slice(4 * (c % 2), 4 * (c % 2) + 4)
        w_all = np.ascontiguousarray(
            np.stack([warr(Wq, hs), warr(Wk, hs), warr(Wv, hs), warr(Wo, hs)], 1)
        )  # [e_in, 4, h, e_out]
        maps.append(
            {
                "qT": np.ascontiguousarray(q[b].T).astype(np.float16),
                "W": w_all,
                "bq": np.ascontiguousarray(bq[hs].T),
            }
        )
    return maps


def kernel(**inputs):
    nc = build_program()
    maps = _in_maps(inputs)
    res = run_bass_kernel_spmd(nc, maps, core_ids=list(range(8)))
    bo = np.asarray(inputs["bo"], dtype=np.float32)
    bv = np.asarray(inputs["bv"], dtype=np.float32)
    Wo = np.asarray(inputs["Wo"], dtype=np.float32).reshape(H, E, E)
    # V-bias contribution folded out of the device kernel:
    # sum_h softmax(..)@ (qWv + bv) @ Wo_h = device_partials + sum_h bv_h @ Wo_h
    bo_eff = bo + np.einsum("he,hef->f", bv, Wo).astype(np.float32)
    out = np.empty((B, S, E), dtype=np.float32)
    for b in range(B):
        part = res.results[2 * b]["out"] + res.results[2 * b + 1]["out"]
        out[b] = part.T + bo_eff
    return out


# revision 45
# speedup vs baseline: 1.0176x; 1.0176x over previous
"""Multi-head attention TRN2 kernel (B=4, S=2048, E=128, H=8) on 8 NeuronCores.

Sharding: core c handles batch b = c // 2 and head group g = c % 2
(heads 4g .. 4g+3).  Each core computes the partial output
outT_partial[e_out, s] = sum_{h in group} (softmax(QK^T/sqrt(E)) V)_h @ Wo_h
for its batch, transposed.  Host sums the two head-group partials per batch,
transposes, and adds bo (plus the host-folded bv and bk contributions).

v2 design (vs v1 baseline at ~204 us):
  - host sends q pre-transposed as f16 [E, S]: no PE transposes, no qT casts
  - bk dropped entirely (adds a per-query constant to scores -> cancels in
    softmax); bv folded into the host-side output bias (as before)
  - attn weights (exp of scores) written by ScalarE directly as fp8e4;
    V projection cast to fp8e4 -> AV matmul and the softmax-denominator
    matmul both run in DoubleRow mode (contract 2 t-blocks of 128 per
    matmul), halving PE time for those stages
  - denominator computed as a tail burst of 8 DoubleRow matmuls with an
    fp8 ones matrix (no DVE fold tree at all)
  - PSUM: sc [128,1024] x2bufs (4 banks) + zts 2 banks + 2 rotating work
    banks (proj / V / dns / outproj)
  - next head's Q/K projections and the V projection are drip-fed into the
    tb loop (2 ops per iteration) so PE work overlaps the exp stream
"""

import sys

for _p in ("/opt/trn_rl_repo",):
    if _p not in sys.path:
        sys.path.insert(0, _p)

import numpy as np

import concourse.bass as bass
import concourse.mybir as mybir
import concourse.tile as tile
from concourse.bass_utils import run_bass_kernel_spmd

F32 = mybir.dt.float32
F16 = mybir.dt.float16
F8 = mybir.dt.float8e4
DR = mybir.MatmulPerfMode.DoubleRow
EXP = mybir.ActivationFunctionType.Exp

B, S, E, H = 4, 2048, 128, 8
NH = 4          # heads per core
TB = S // 128   # 16 t blocks
SW = 1024       # s-half width
NC = 512        # psum-bank chunk
SCALE = 1.0 / np.sqrt(E)

_prog_cache = {}


def build_program():
    if "nc" in _prog_cache:
        return _prog_cache["nc"]

    import concourse.bacc as bacc

    nc = bacc.Bacc("TRN2", target_bir_lowering=False, debug=False)

    qt_d = nc.dram_tensor("qT", [E, S], F16, kind="ExternalInput").ap()
    # packed weights: dim1 = (Wq, Wk, Wv, Wo); 4KB DRAM rows for DMA speed
    w_d = nc.dram_tensor("W", [E, 4, NH, E], F16, kind="ExternalInput").ap()
    bq_d = nc.dram_tensor("bq", [E, NH], F32, kind="ExternalInput").ap()
    out_d = nc.dram_tensor("out", [E, S], F32, kind="ExternalOutput").ap()

    with tile.TileContext(nc) as tc:
        _emit(nc, tc, qt_d, w_d, bq_d, out_d)

    nc.compile()
    _prog_cache["nc"] = nc
    return nc


def _emit(nc, tc, qt_d, w_d, bq_d, out_d):
    from collections import deque
    from contextlib import ExitStack

    ctx = ExitStack()
    consts = ctx.enter_context(tc.tile_pool(name="consts", bufs=1))
    heads = ctx.enter_context(tc.tile_pool(name="heads", bufs=2))
    attns = ctx.enter_context(tc.tile_pool(name="attns", bufs=2))
    works = ctx.enter_context(tc.tile_pool(name="works", bufs=2))
    psum_sc = ctx.enter_context(tc.tile_pool(name="psum_sc", bufs=2, space="PSUM"))
    psum_av = ctx.enter_context(tc.tile_pool(name="psum_av", bufs=2, space="PSUM"))
    psum_wk = ctx.enter_context(tc.tile_pool(name="psum_wk", bufs=2, space="PSUM"))

    # ---- constants / inputs ----
    # DMA issue cost is ~650ns per call per engine queue and per-queue
    # bandwidth is poor for small rows; use few, large-row transfers split
    # across the two HWDGE issue queues (Sync + ScalarE), ordered by use.
    qT = consts.tile([128, S], F16, tag="qT")  # [e, s]
    nc.sync.dma_start(out=qT, in_=qt_d)  # one transfer: 4KB rows ~3x faster
    w_all4 = consts.tile([128, 4, NH, 128], F16, tag="w_all4")
    nc.scalar.dma_start(out=w_all4[:, 0:2], in_=w_d[:, 0:2])  # Wq+Wk first
    bq = consts.tile([128, NH], F32, tag="bq")  # [f, h]
    nc.scalar.dma_start(out=bq, in_=bq_d)
    nc.scalar.dma_start(out=w_all4[:, 2:4], in_=w_d[:, 2:4])
    wq = w_all4[:, 0]  # [e_in, h, e_out]
    wk = w_all4[:, 1]
    wv = w_all4[:, 2]
    wo = w_all4[:, 3]  # [f, h, g]

    ones8 = consts.tile([128, 2, 128], F8, tag="ones8")
    nc.vector.memset(ones8, 1.0)

    v8 = consts.tile([128, TB, NH * 128], F8, tag="v8")  # [t_sub, tb, (h f)]
    wv_all = wv.rearrange("e h f -> e (h f)")

    acc_a = [
        consts.tile([128, SW], F32, tag=f"acca{s}", name=f"acca{s}") for s in range(2)
    ]
    acc_b = [
        consts.tile([128, SW], F32, tag=f"accb{s}", name=f"accb{s}") for s in range(2)
    ]

    def v_block(tb):
        ps = psum_wk.tile([128, NC], F32, tag="work", name=f"vps_{tb}")
        nc.tensor.matmul(ps, lhsT=qT[:, tb * 128 : (tb + 1) * 128], rhs=wv_all,
                         start=True, stop=True)
        nc.vector.tensor_copy(v8[:, tb, :], ps)

    def proj_q(h, qt_h, j):
        ps = psum_wk.tile([128, NC], F32, tag="work", name=f"qp{h}_{j}")
        nc.tensor.matmul(ps, lhsT=wq[:, h, :], rhs=qT[:, j * 512 : (j + 1) * 512],
                         start=True, stop=True)
        nc.vector.tensor_scalar_add(qt_h[:, j * 512 : (j + 1) * 512], ps,
                                    bq[:, h : h + 1])

    def proj_k(h, kt_h, j):
        ps = psum_wk.tile([128, NC], F32, tag="work", name=f"kp{h}_{j}")
        nc.tensor.matmul(ps, lhsT=wk[:, h, :], rhs=qT[:, j * 512 : (j + 1) * 512],
                         start=True, stop=True)
        nc.vector.tensor_copy(kt_h[:, j * 512 : (j + 1) * 512], ps)

    def alloc_head(h):
        qt_h = heads.tile([128, S], F16, tag="QT", name=f"qt{h}")  # [f, s]
        kt_h = heads.tile([128, S], F16, tag="KT", name=f"kt{h}")  # [f, t]
        return qt_h, kt_h

    # prologue: only what the first scores matmul needs, evacuated on the
    # (otherwise idle) ScalarE so the DVE isn't on the critical path
    cur = alloc_head(0)
    ps = psum_wk.tile([128, NC], F32, tag="work", name="qp0_0p")
    nc.tensor.matmul(ps, lhsT=wq[:, 0, :], rhs=qT[:, 0:512], start=True, stop=True)
    nc.scalar.add(cur[0][:, 0:512], ps, bq[:, 0:1])
    ps = psum_wk.tile([128, NC], F32, tag="work", name="kp0_0p")
    nc.tensor.matmul(ps, lhsT=wk[:, 0, :], rhs=qT[:, 0:512], start=True, stop=True)
    nc.vector.tensor_copy(cur[1][:, 0:512], ps)  # DVE, parallel to ScalarE adds
    ps = psum_wk.tile([128, NC], F32, tag="work", name="qp0_1p")
    nc.tensor.matmul(ps, lhsT=wq[:, 0, :], rhs=qT[:, 512:1024], start=True, stop=True)
    nc.scalar.add(cur[0][:, 512:1024], ps, bq[:, 0:1])

    osb_tiles = {}

    def out_proj(h, sh, c, ztn):
        # output projection + head accumulation for chunk c of segment (h, sh)
        wo_ps = psum_wk.tile([128, NC], F32, tag="work", name=f"wop{h}{sh}{c}")
        nc.tensor.matmul(wo_ps, lhsT=wo[:, h, :], rhs=ztn, start=True, stop=True)
        asl = slice(c * 512, (c + 1) * 512)
        if h == 0:
            nc.vector.tensor_copy(acc_a[sh][:, asl], wo_ps)
        elif h == 1:
            nc.vector.tensor_add(acc_b[sh][:, asl], acc_a[sh][:, asl], wo_ps)
        elif h == 2:
            nc.vector.tensor_add(acc_a[sh][:, asl], acc_b[sh][:, asl], wo_ps)
        else:
            if sh not in osb_tiles:
                osb_tiles[sh] = works.tile([128, SW], F32, tag="osb",
                                           name=f"osb{sh}")
            osb = osb_tiles[sh]
            nc.vector.tensor_add(osb[:, asl], acc_a[sh][:, asl], wo_ps)
            if c == 1:
                ssl = slice(sh * SW, (sh + 1) * SW)
                nc.sync.dma_start(out=out_d[:, ssl], in_=osb)

    nxt = None
    deferred = None  # tail work from the previous segment
    for h in range(NH):
        qt_h, kt_h = cur
        for sh in range(2):
            # per-iteration emission schedule. pre[tb] runs at the top of
            # iteration tb (projection drip / deferred out-proj: deps are
            # already met, so they never head-block the FIFO); post[tb] runs
            # after scores+exp of iteration tb (AV/dns pairs whose exp dep
            # clears exactly when the PE FIFO reaches them).
            pre = [[] for _ in range(TB + 1)]
            post = [[] for _ in range(TB + 1)]

            def place(tb, thunk):
                pre[min(tb, TB)].append(thunk)

            def place_post(tb, thunk):
                post[min(tb, TB)].append(thunk)

            # tail of the previous segment: its last matmuls go after this
            # segment's sc(1) (same exp gate — sc(1) must issue first); then
            # per-chunk chains norm0 / wo0 / norm1 / wo1 so chunk 0's work
            # bank recycles as early as possible.
            # previous segment's tail, then its normalization, then its
            # out-projection, then this segment's projection drip, then the
            # denominator accumulation — strictly in that order so the two
            # rotating work banks never force a FIFO head-block.
            base = 0
            if deferred is not None:
                n_mms = len(deferred["mms"])
                for i, t in enumerate(deferred["mms"]):
                    place_post(1 + i // 2, t)
                base = 1 + (n_mms + 1) // 2
                place_post(base, deferred["norm"][0])
                place_post(base + 1, deferred["wo"][0])
                place_post(base + 1, deferred["norm"][1])
                place_post(base + 2, deferred["wo"][1])
                deferred = None

            drip = deque()
            drip0 = 0
            if h == 0 and sh == 0:
                vb = lambda t: (lambda tt=t: v_block(tt))
                pk = lambda j: (lambda jj=j: proj_k(0, kt_h, jj))
                pq = lambda j: (lambda jj=j: proj_q(0, qt_h, jj))
                drip += [vb(0), vb(1), pk(1), vb(2), vb(3), pk(2), vb(4), vb(5),
                         pq(2), vb(6), vb(7), pk(3), vb(8), vb(9), pq(3)]
                drip += [vb(t) for t in range(10, TB)]
            if sh == 1 and h + 1 < NH:
                nxt = alloc_head(h + 1)
                hh, nq, nk = h + 1, nxt[0], nxt[1]
                drip += [lambda j=j: proj_q(hh, nq, j) for j in range(4)]
                drip += [lambda j=j: proj_k(hh, nk, j) for j in range(4)]
                drip0 = 4
            n_drip = len(drip)
            for i, t in enumerate(drip):
                place_post(drip0 + i // 2, t)
            first_free = max(drip0 + (n_drip + 1) // 2, 5, base + 3)

            s0 = sh * SW
            attnT = attns.tile([128, TB, SW], F8, tag="attnT", name=f"at{h}{sh}")
            zts = [
                psum_av.tile([128, NC], F32, tag="zt", name=f"zt{h}{sh}{c}")
                for c in range(2)
            ]
            dn_tiles = {}

            def dns_pair(p, c, at=attnT, dn=dn_tiles, hh=h, ss=sh):
                if c not in dn:
                    dn[c] = psum_wk.tile(
                        [128, NC], F32, tag="work", name=f"dn{hh}{ss}{c}"
                    )
                nc.tensor.matmul(
                    dn[c],
                    lhsT=ones8,
                    rhs=at[:, 2 * p : 2 * p + 2, c * 512 : (c + 1) * 512],
                    start=(p == 0), stop=(p == 7), perf_mode=DR,
                )

            def av_pair(p, c, at=attnT, z=zts, hh=h):
                nc.tensor.matmul(
                    z[c],
                    lhsT=v8[:, 2 * p : 2 * p + 2, hh * 128 : (hh + 1) * 128],
                    rhs=at[:, 2 * p : 2 * p + 2, c * 512 : (c + 1) * 512],
                    start=(p == 0), stop=(p == 7), perf_mode=DR,
                )

            for p in range(8):
                place_post(
                    max(2 * p + 3, first_free + p),
                    lambda p=p, f=dns_pair: (f(p, 0), f(p, 1)),
                )
                place_post(
                    2 * p + 3, lambda p=p, f=av_pair: (f(p, 0), f(p, 1))
                )

            for tb in range(TB):
                for t in pre[tb]:
                    t()
                sc = psum_sc.tile([128, SW], F32, tag="sc", name=f"sc{h}{sh}{tb}")
                for c in range(2):
                    nc.tensor.matmul(
                        sc[:, c * 512 : (c + 1) * 512],
                        lhsT=kt_h[:, tb * 128 : (tb + 1) * 128],
                        rhs=qt_h[:, s0 + c * 512 : s0 + (c + 1) * 512],
                        start=True, stop=True,
                    )
                nc.scalar.activation(attnT[:, tb, :], sc, EXP, scale=SCALE)
                for t in post[tb]:
                    t()

            holder = {}

            def norm_chunk(c, dn=dn_tiles, z=zts, hol=holder, hh=h, ss=sh):
                recip = works.tile([128, NC], F32, tag="recip",
                                   name=f"rc{hh}{ss}{c}")
                nc.vector.reciprocal_approx_fast(recip, dn[c])
                ztn = works.tile([128, NC], F16, tag="ztn", name=f"zn{hh}{ss}{c}")
                nc.vector.tensor_mul(ztn, z[c], recip)
                hol[c] = ztn

            if h == NH - 1 and sh == 1:
                # final segment: drain immediately, complete per-chunk chains
                for t in pre[TB]:
                    t()
                for t in post[TB]:
                    t()
                for c in range(2):
                    norm_chunk(c)
                    out_proj(h, sh, c, holder[c])
            else:
                deferred = {
                    "mms": pre[TB] + post[TB],
                    "norm": [lambda c=c, f=norm_chunk: f(c) for c in range(2)],
                    "wo": [
                        lambda c=c, hol=holder, hh=h, ss=sh: out_proj(
                            hh, ss, c, hol[c]
                        )
                        for c in range(2)
                    ],
                }
        if h + 1 < NH:
            cur = nxt

    ctx.close()


def _in_maps(inputs):
    q = np.asarray(inputs["q"], dtype=np.float32)
    Wq = np.asarray(inputs["Wq"], dtype=np.float32)
    bq = np.asarray(inputs["bq"], dtype=np.float32)
    Wk = np.asarray(inputs["Wk"], dtype=np.float32)
    Wv = np.asarray(inputs["Wv"], dtype=np.float32)
    Wo = np.asarray(inputs["Wo"], dtype=np.float32).reshape(H, E, E)

    def warr(w, hs):  # [h, e_in, e_out] slice -> [e_in, h, e_out] f16
        return w[hs].transpose(1, 0, 2).astype(np.float16)

    maps = []
    for c in range(8):
        b = c // 2
        hs = slice(4 * (c % 2), 4 * (c % 2) + 4)
        w_all = np.ascontiguousarray(
            np.stack([warr(Wq, hs), warr(Wk, hs), warr(Wv, hs), warr(Wo, hs)], 1)
        )  # [e_in, 4, h, e_out]
        maps.append(
            {
                "qT": np.ascontiguousarray(q[b].T).astype(np.float16),
                "W": w_all,
                "bq": np.ascontiguousarray(bq[hs].T),
            }
        )
    return maps


def kernel(**inputs):
    nc = build_program()
    maps = _in_maps(inputs)
    res = run_bass_kernel_spmd(nc, maps, core_ids=list(range(8)))
    bo = np.asarray(inputs["bo"], dtype=np.float32)
    bv = np.asarray(inputs["bv"], dtype=np.float32)
    Wo = np.asarray(inputs["Wo"], dtype=np.float32).reshape(H, E, E)
    # V-bias contribution folded out of the device kernel:
    # sum_h softmax(..)@ (qWv + bv) @ Wo_h = device_partials + sum_h bv_h @ Wo_h
    bo_eff = bo + np.einsum("he,hef->f", bv, Wo).astype(np.float32)
    out = np.empty((B, S, E), dtype=np.float32)
    for b in range(B):
        part = res.results[2 * b]["out"] + res.results[2 * b + 1]["out"]
        out[b] = part.T + bo_eff
    return out


# revision 47
# speedup vs baseline: 1.0197x; 1.0021x over previous
"""Multi-head attention TRN2 kernel (B=4, S=2048, E=128, H=8) on 8 NeuronCores.

Sharding: core c handles batch b = c // 2 and head group g = c % 2
(heads 4g .. 4g+3).  Each core computes the partial output
outT_partial[e_out, s] = sum_{h in group} (softmax(QK^T/sqrt(E)) V)_h @ Wo_h
for its batch, transposed.  Host sums the two head-group partials per batch,
transposes, and adds bo (plus the host-folded bv and bk contributions).

v2 design (vs v1 baseline at ~204 us):
  - host sends q pre-transposed as f16 [E, S]: no PE transposes, no qT casts
  - bk dropped entirely (adds a per-query constant to scores -> cancels in
    softmax); bv folded into the host-side output bias (as before)
  - attn weights (exp of scores) written by ScalarE directly as fp8e4;
    V projection cast to fp8e4 -> AV matmul and the softmax-denominator
    matmul both run in DoubleRow mode (contract 2 t-blocks of 128 per
    matmul), halving PE time for those stages
  - denominator computed as a tail burst of 8 DoubleRow matmuls with an
    fp8 ones matrix (no DVE fold tree at all)
  - PSUM: sc [128,1024] x2bufs (4 banks) + zts 2 banks + 2 rotating work
    banks (proj / V / dns / outproj)
  - next head's Q/K projections and the V projection are drip-fed into the
    tb loop (2 ops per iteration) so PE work overlaps the exp stream
"""

import sys

for _p in ("/opt/trn_rl_repo",):
    if _p not in sys.path:
        sys.path.insert(0, _p)

import numpy as np

import concourse.bass as bass
import concourse.mybir as mybir
import concourse.tile as tile
from concourse.bass_utils import run_bass_kernel_spmd

F32 = mybir.dt.float32
F16 = mybir.dt.float16
F8 = mybir.dt.float8e4
DR = mybir.MatmulPerfMode.DoubleRow
EXP = mybir.ActivationFunctionType.Exp

B, S, E, H = 4, 2048, 128, 8
NH = 4          # heads per core
TB = S // 128   # 16 t blocks
SW = 1024       # s-half width
NC = 512        # psum-bank chunk
SCALE = 1.0 / np.sqrt(E)

_prog_cache = {}


def build_program():
    if "nc" in _prog_cache:
        return _prog_cache["nc"]

    import concourse.bacc as bacc

    nc = bacc.Bacc("TRN2", target_bir_lowering=False, debug=False)

    qt_d = nc.dram_tensor("qT", [E, S], F16, kind="ExternalInput").ap()
    # packed weights: dim1 = (Wq, Wk, Wv, Wo); 4KB DRAM rows for DMA speed
    w_d = nc.dram_tensor("W", [E, 4, NH, E], F16, kind="ExternalInput").ap()
    bq_d = nc.dram_tensor("bq", [E, NH], F32, kind="ExternalInput").ap()
    out_d = nc.dram_tensor("out", [E, S], F32, kind="ExternalOutput").ap()

    with tile.TileContext(nc) as tc:
        _emit(nc, tc, qt_d, w_d, bq_d, out_d)

    nc.compile()
    _prog_cache["nc"] = nc
    return nc


def _emit(nc, tc, qt_d, w_d, bq_d, out_d):
    from collections import deque
    from contextlib import ExitStack

    ctx = ExitStack()
    consts = ctx.enter_context(tc.tile_pool(name="consts", bufs=1))
    heads = ctx.enter_context(tc.tile_pool(name="heads", bufs=2))
    attns = ctx.enter_context(tc.tile_pool(name="attns", bufs=2))
    works = ctx.enter_context(tc.tile_pool(name="works", bufs=2))
    psum_sc = ctx.enter_context(tc.tile_pool(name="psum_sc", bufs=2, space="PSUM"))
    psum_av = ctx.enter_context(tc.tile_pool(name="psum_av", bufs=2, space="PSUM"))
    psum_wk = ctx.enter_context(tc.tile_pool(name="psum_wk", bufs=2, space="PSUM"))

    # ---- constants / inputs ----
    # DMA issue cost is ~650ns per call per engine queue and per-queue
    # bandwidth is poor for small rows; use few, large-row transfers split
    # across the two HWDGE issue queues (Sync + ScalarE), ordered by use.
    qT = consts.tile([128, S], F16, tag="qT")  # [e, s]
    nc.sync.dma_start(out=qT, in_=qt_d)  # one transfer: 4KB rows ~3x faster
    w_all4 = consts.tile([128, 4, NH, 128], F16, tag="w_all4")
    nc.scalar.dma_start(out=w_all4[:, 0:2], in_=w_d[:, 0:2])  # Wq+Wk first
    bq = consts.tile([128, NH], F32, tag="bq")  # [f, h]
    nc.scalar.dma_start(out=bq, in_=bq_d)
    nc.scalar.dma_start(out=w_all4[:, 2:4], in_=w_d[:, 2:4])
    wq = w_all4[:, 0]  # [e_in, h, e_out]
    wk = w_all4[:, 1]
    wv = w_all4[:, 2]
    wo = w_all4[:, 3]  # [f, h, g]

    ones8 = consts.tile([128, 2, 128], F8, tag="ones8")
    nc.vector.memset(ones8, 1.0)

    v8 = consts.tile([128, TB, NH * 128], F8, tag="v8")  # [t_sub, tb, (h f)]
    wv_all = wv.rearrange("e h f -> e (h f)")

    acc_a = [
        consts.tile([128, SW], F32, tag=f"acca{s}", name=f"acca{s}") for s in range(2)
    ]
    acc_b = [
        consts.tile([128, SW], F32, tag=f"accb{s}", name=f"accb{s}") for s in range(2)
    ]

    def v_block(tb, pool=None, tag="work"):
        pool = pool or psum_wk
        ps = pool.tile([128, NC], F32, tag=tag, name=f"vps_{tb}")
        nc.tensor.matmul(ps, lhsT=qT[:, tb * 128 : (tb + 1) * 128], rhs=wv_all,
                         start=True, stop=True)
        nc.vector.tensor_copy(v8[:, tb, :], ps)

    def proj_q(h, qt_h, j):
        ps = psum_wk.tile([128, NC], F32, tag="work", name=f"qp{h}_{j}")
        nc.tensor.matmul(ps, lhsT=wq[:, h, :], rhs=qT[:, j * 512 : (j + 1) * 512],
                         start=True, stop=True)
        nc.vector.tensor_scalar_add(qt_h[:, j * 512 : (j + 1) * 512], ps,
                                    bq[:, h : h + 1])

    def proj_k(h, kt_h, j):
        ps = psum_wk.tile([128, NC], F32, tag="work", name=f"kp{h}_{j}")
        nc.tensor.matmul(ps, lhsT=wk[:, h, :], rhs=qT[:, j * 512 : (j + 1) * 512],
                         start=True, stop=True)
        nc.vector.tensor_copy(kt_h[:, j * 512 : (j + 1) * 512], ps)

    def alloc_head(h):
        qt_h = heads.tile([128, S], F16, tag="QT", name=f"qt{h}")  # [f, s]
        kt_h = heads.tile([128, S], F16, tag="KT", name=f"kt{h}")  # [f, t]
        return qt_h, kt_h

    # prologue: only what the first scores matmul needs, evacuated on the
    # (otherwise idle) ScalarE so the DVE isn't on the critical path
    cur = alloc_head(0)
    ps = psum_wk.tile([128, NC], F32, tag="work", name="qp0_0p")
    nc.tensor.matmul(ps, lhsT=wq[:, 0, :], rhs=qT[:, 0:512], start=True, stop=True)
    nc.scalar.add(cur[0][:, 0:512], ps, bq[:, 0:1])
    ps = psum_wk.tile([128, NC], F32, tag="work", name="kp0_0p")
    nc.tensor.matmul(ps, lhsT=wk[:, 0, :], rhs=qT[:, 0:512], start=True, stop=True)
    nc.vector.tensor_copy(cur[1][:, 0:512], ps)  # DVE, parallel to ScalarE adds
    ps = psum_wk.tile([128, NC], F32, tag="work", name="qp0_1p")
    nc.tensor.matmul(ps, lhsT=wq[:, 0, :], rhs=qT[:, 512:1024], start=True, stop=True)
    nc.scalar.add(cur[0][:, 512:1024], ps, bq[:, 0:1])

    osb_tiles = {}

    def out_proj(h, sh, c, ztn):
        # output projection + head accumulation for chunk c of segment (h, sh)
        wo_ps = psum_wk.tile([128, NC], F32, tag="work", name=f"wop{h}{sh}{c}")
        nc.tensor.matmul(wo_ps, lhsT=wo[:, h, :], rhs=ztn, start=True, stop=True)
        asl = slice(c * 512, (c + 1) * 512)
        if h == 0:
            nc.vector.tensor_copy(acc_a[sh][:, asl], wo_ps)
        elif h == 1:
            nc.vector.tensor_add(acc_b[sh][:, asl], acc_a[sh][:, asl], wo_ps)
        elif h == 2:
            nc.vector.tensor_add(acc_a[sh][:, asl], acc_b[sh][:, asl], wo_ps)
        else:
            if sh not in osb_tiles:
                osb_tiles[sh] = works.tile([128, SW], F32, tag="osb",
                                           name=f"osb{sh}")
            osb = osb_tiles[sh]
            nc.vector.tensor_add(osb[:, asl], acc_a[sh][:, asl], wo_ps)
            if c == 1:
                ssl = slice(sh * SW, (sh + 1) * SW)
                nc.sync.dma_start(out=out_d[:, ssl], in_=osb)

    nxt = None
    deferred = None  # tail work from the previous segment
    for h in range(NH):
        qt_h, kt_h = cur
        for sh in range(2):
            # per-iteration emission schedule. pre[tb] runs at the top of
            # iteration tb (projection drip / deferred out-proj: deps are
            # already met, so they never head-block the FIFO); post[tb] runs
            # after scores+exp of iteration tb (AV/dns pairs whose exp dep
            # clears exactly when the PE FIFO reaches them).
            pre = [[] for _ in range(TB + 1)]
            post = [[] for _ in range(TB + 1)]

            def place(tb, thunk):
                pre[min(tb, TB)].append(thunk)

            def place_post(tb, thunk):
                post[min(tb, TB)].append(thunk)

            # tail of the previous segment: its last matmuls go after this
            # segment's sc(1) (same exp gate — sc(1) must issue first); then
            # per-chunk chains norm0 / wo0 / norm1 / wo1 so chunk 0's work
            # bank recycles as early as possible.
            # previous segment's tail, then its normalization, then its
            # out-projection, then this segment's projection drip, then the
            # denominator accumulation — strictly in that order so the two
            # rotating work banks never force a FIFO head-block.
            base = 0
            if deferred is not None:
                n_mms = len(deferred["mms"])
                for i, t in enumerate(deferred["mms"]):
                    place_post(1 + i // 2, t)
                base = 1 + (n_mms + 1) // 2
                place_post(base, deferred["norm"][0])
                place_post(base + 1, deferred["wo"][0])
                place_post(base + 1, deferred["norm"][1])
                place_post(base + 2, deferred["wo"][1])
                deferred = None

            drip = deque()
            drip0 = 0
            if h == 0 and sh == 0:
                vb = lambda t: (lambda tt=t: v_block(tt))
                pk = lambda j: (lambda jj=j: proj_k(0, kt_h, jj))
                pq = lambda j: (lambda jj=j: proj_q(0, qt_h, jj))
                # V0/V1 borrow the (idle until slot 3) zts banks so they
                # don't wait on the prologue's work-bank evacuations
                vz = lambda t: (lambda tt=t: v_block(tt, psum_av, "zt"))
                drip += [vz(0), vz(1), pk(1), vb(2), vb(3), pk(2), vb(4), vb(5),
                         pq(2), vb(6), vb(7), pk(3), vb(8), vb(9), pq(3)]
                drip += [vb(t) for t in range(10, TB)]
            if sh == 1 and h + 1 < NH:
                nxt = alloc_head(h + 1)
                hh, nq, nk = h + 1, nxt[0], nxt[1]
                drip += [lambda j=j: proj_q(hh, nq, j) for j in range(4)]
                drip += [lambda j=j: proj_k(hh, nk, j) for j in range(4)]
                drip0 = 4
            n_drip = len(drip)
            for i, t in enumerate(drip):
                place_post(drip0 + i // 2, t)
            first_free = max(drip0 + (n_drip + 1) // 2, 5, base + 3)

            s0 = sh * SW
            attnT = attns.tile([128, TB, SW], F8, tag="attnT", name=f"at{h}{sh}")
            zts = [
                psum_av.tile([128, NC], F32, tag="zt", name=f"zt{h}{sh}{c}")
                for c in range(2)
            ]
            dn_tiles = {}

            def dns_pair(p, c, at=attnT, dn=dn_tiles, hh=h, ss=sh):
                if c not in dn:
                    dn[c] = psum_wk.tile(
                        [128, NC], F32, tag="work", name=f"dn{hh}{ss}{c}"
                    )
                nc.tensor.matmul(
                    dn[c],
                    lhsT=ones8,
                    rhs=at[:, 2 * p : 2 * p + 2, c * 512 : (c + 1) * 512],
                    start=(p == 0), stop=(p == 7), perf_mode=DR,
                )

            def av_pair(p, c, at=attnT, z=zts, hh=h):
                nc.tensor.matmul(
                    z[c],
                    lhsT=v8[:, 2 * p : 2 * p + 2, hh * 128 : (hh + 1) * 128],
                    rhs=at[:, 2 * p : 2 * p + 2, c * 512 : (c + 1) * 512],
                    start=(p == 0), stop=(p == 7), perf_mode=DR,
                )

            for p in range(8):
                place_post(
                    max(2 * p + 3, first_free + p),
                    lambda p=p, f=dns_pair: (f(p, 0), f(p, 1)),
                )
                place_post(
                    2 * p + 3, lambda p=p, f=av_pair: (f(p, 0), f(p, 1))
                )

            for tb in range(TB):
                for t in pre[tb]:
                    t()
                sc = psum_sc.tile([128, SW], F32, tag="sc", name=f"sc{h}{sh}{tb}")
                for c in range(2):
                    nc.tensor.matmul(
                        sc[:, c * 512 : (c + 1) * 512],
                        lhsT=kt_h[:, tb * 128 : (tb + 1) * 128],
                        rhs=qt_h[:, s0 + c * 512 : s0 + (c + 1) * 512],
                        start=True, stop=True,
                    )
                nc.scalar.activation(attnT[:, tb, :], sc, EXP, scale=SCALE)
                for t in post[tb]:
                    t()

            holder = {}

            def norm_chunk(c, dn=dn_tiles, z=zts, hol=holder, hh=h, ss=sh):
                recip = works.tile([128, NC], F32, tag="recip",
                                   name=f"rc{hh}{ss}{c}")
                nc.vector.reciprocal_approx_fast(recip, dn[c])
                ztn = works.tile([128, NC], F16, tag="ztn", name=f"zn{hh}{ss}{c}")
                nc.vector.tensor_mul(ztn, z[c], recip)
                hol[c] = ztn

            if h == NH - 1 and sh == 1:
                # final segment: drain immediately, complete per-chunk chains
                for t in pre[TB]:
                    t()
                for t in post[TB]:
                    t()
                for c in range(2):
                    norm_chunk(c)
                    out_proj(h, sh, c, holder[c])
            else:
                deferred = {
                    "mms": pre[TB] + post[TB],
                    "norm": [lambda c=c, f=norm_chunk: f(c) for c in range(2)],
                    "wo": [
                        lambda c=c, hol=holder, hh=h, ss=sh: out_proj(
                            hh, ss, c, hol[c]
                        )
                        for c in range(2)
                    ],
                }
        if h + 1 < NH:
            cur = nxt

    ctx.close()


def _in_maps(inputs):
    q = np.asarray(inputs["q"], dtype=np.float32)
    Wq = np.asarray(inputs["Wq"], dtype=np.float32)
    bq = np.asarray(inputs["bq"], dtype=np.float32)
    Wk = np.asarray(inputs["Wk"], dtype=np.float32)
    Wv = np.asarray(inputs["Wv"], dtype=np.float32)
    Wo = np.asarray(inputs["Wo"], dtype=np.float32).reshape(H, E, E)

    def warr(w, hs):  # [h, e_in, e_out] slice -> [e_in, h, e_out] f16
        return w[hs].transpose(1, 0, 2).astype(np.float16)

    maps = []
    for c in range(8):
        b = c // 2
        hs = slice(4 * (c % 2), 4 * (c % 2) + 4)
        w_all = np.ascontiguousarray(
            np.stack([warr(Wq, hs), warr(Wk, hs), warr(Wv, hs), warr(Wo, hs)], 1)
        )  # [e_in, 4, h, e_out]
        maps.append(
            {
                "qT": np.ascontiguousarray(q[b].T).astype(np.float16),
                "W": w_all,
                "bq": np.ascontiguousarray(bq[hs].T),
            }
        )
    return maps


def kernel(**inputs):
    nc = build_program()
    maps = _in_maps(inputs)
    res = run_bass_kernel_spmd(nc, maps, core_ids=list(range(8)))
    bo = np.asarray(inputs["bo"], dtype=np.float32)
    bv = np.asarray(inputs["bv"], dtype=np.float32)
    Wo = np.asarray(inputs["Wo"], dtype=np.float32).reshape(H, E, E)
    # V-bias contribution folded out of the device kernel:
    # sum_h softmax(..)@ (qWv + bv) @ Wo_h = device_partials + sum_h bv_h @ Wo_h
    bo_eff = bo + np.einsum("he,hef->f", bv, Wo).astype(np.float32)
    out = np.empty((B, S, E), dtype=np.float32)
    for b in range(B):
        part = res.results[2 * b]["out"] + res.results[2 * b + 1]["out"]
        out[b] = part.T + bo_eff
    return out


# revision 48
# speedup vs baseline: 1.0221x; 1.0024x over previous
"""Multi-head attention TRN2 kernel (B=4, S=2048, E=128, H=8) on 8 NeuronCores.

Sharding: core c handles batch b = c // 2 and head group g = c % 2
(heads 4g .. 4g+3).  Each core computes the partial output
outT_partial[e_out, s] = sum_{h in group} (softmax(QK^T/sqrt(E)) V)_h @ Wo_h
for its batch, transposed.  Host sums the two head-group partials per batch,
transposes, and adds bo (plus the host-folded bv and bk contributions).

v2 design (vs v1 baseline at ~204 us):
  - host sends q pre-transposed as f16 [E, S]: no PE transposes, no qT casts
  - bk dropped entirely (adds a per-query constant to scores -> cancels in
    softmax); bv folded into the host-side output bias (as before)
  - attn weights (exp of scores) written by ScalarE directly as fp8e4;
    V projection cast to fp8e4 -> AV matmul and the softmax-denominator
    matmul both run in DoubleRow mode (contract 2 t-blocks of 128 per
    matmul), halving PE time for those stages
  - denominator computed as a tail burst of 8 DoubleRow matmuls with an
    fp8 ones matrix (no DVE fold tree at all)
  - PSUM: sc [128,1024] x2bufs (4 banks) + zts 2 banks + 2 rotating work
    banks (proj / V / dns / outproj)
  - next head's Q/K projections and the V projection are drip-fed into the
    tb loop (2 ops per iteration) so PE work overlaps the exp stream
"""

import sys

for _p in ("/opt/trn_rl_repo",):
    if _p not in sys.path:
        sys.path.insert(0, _p)

import numpy as np

import concourse.bass as bass
import concourse.mybir as mybir
import concourse.tile as tile
from concourse.bass_utils import run_bass_kernel_spmd

F32 = mybir.dt.float32
F16 = mybir.dt.float16
F8 = mybir.dt.float8e4
DR = mybir.MatmulPerfMode.DoubleRow
EXP = mybir.ActivationFunctionType.Exp

B, S, E, H = 4, 2048, 128, 8
NH = 4          # heads per core
TB = S // 128   # 16 t blocks
SW = 1024       # s-half width
NC = 512        # psum-bank chunk
SCALE = 1.0 / np.sqrt(E)

_prog_cache = {}


def build_program():
    if "nc" in _prog_cache:
        return _prog_cache["nc"]

    import concourse.bacc as bacc

    nc = bacc.Bacc("TRN2", target_bir_lowering=False, debug=False)

    qt_d = nc.dram_tensor("qT", [E, S], F16, kind="ExternalInput").ap()
    # packed weights: dim1 = (Wq, Wk, Wv, Wo); 4KB DRAM rows for DMA speed
    w_d = nc.dram_tensor("W", [E, 4, NH, E], F16, kind="ExternalInput").ap()
    bq_d = nc.dram_tensor("bq", [E, NH], F32, kind="ExternalInput").ap()
    out_d = nc.dram_tensor("out", [E, S], F16, kind="ExternalOutput").ap()

    with tile.TileContext(nc) as tc:
        _emit(nc, tc, qt_d, w_d, bq_d, out_d)

    nc.compile()
    _prog_cache["nc"] = nc
    return nc


def _emit(nc, tc, qt_d, w_d, bq_d, out_d):
    from collections import deque
    from contextlib import ExitStack

    ctx = ExitStack()
    consts = ctx.enter_context(tc.tile_pool(name="consts", bufs=1))
    heads = ctx.enter_context(tc.tile_pool(name="heads", bufs=2))
    attns = ctx.enter_context(tc.tile_pool(name="attns", bufs=2))
    works = ctx.enter_context(tc.tile_pool(name="works", bufs=2))
    psum_sc = ctx.enter_context(tc.tile_pool(name="psum_sc", bufs=2, space="PSUM"))
    psum_av = ctx.enter_context(tc.tile_pool(name="psum_av", bufs=2, space="PSUM"))
    psum_wk = ctx.enter_context(tc.tile_pool(name="psum_wk", bufs=2, space="PSUM"))

    # ---- constants / inputs ----
    # DMA issue cost is ~650ns per call per engine queue and per-queue
    # bandwidth is poor for small rows; use few, large-row transfers split
    # across the two HWDGE issue queues (Sync + ScalarE), ordered by use.
    qT = consts.tile([128, S], F16, tag="qT")  # [e, s]
    nc.sync.dma_start(out=qT, in_=qt_d)  # one transfer: 4KB rows ~3x faster
    w_all4 = consts.tile([128, 4, NH, 128], F16, tag="w_all4")
    nc.scalar.dma_start(out=w_all4[:, 0:2], in_=w_d[:, 0:2])  # Wq+Wk first
    bq = consts.tile([128, NH], F32, tag="bq")  # [f, h]
    nc.scalar.dma_start(out=bq, in_=bq_d)
    nc.scalar.dma_start(out=w_all4[:, 2:4], in_=w_d[:, 2:4])
    wq = w_all4[:, 0]  # [e_in, h, e_out]
    wk = w_all4[:, 1]
    wv = w_all4[:, 2]
    wo = w_all4[:, 3]  # [f, h, g]

    ones8 = consts.tile([128, 2, 128], F8, tag="ones8")
    nc.vector.memset(ones8, 1.0)

    v8 = consts.tile([128, TB, NH * 128], F8, tag="v8")  # [t_sub, tb, (h f)]
    wv_all = wv.rearrange("e h f -> e (h f)")

    acc_a = [
        consts.tile([128, SW], F32, tag=f"acca{s}", name=f"acca{s}") for s in range(2)
    ]
    acc_b = [
        consts.tile([128, SW], F32, tag=f"accb{s}", name=f"accb{s}") for s in range(2)
    ]

    def v_block(tb, pool=None, tag="work"):
        pool = pool or psum_wk
        ps = pool.tile([128, NC], F32, tag=tag, name=f"vps_{tb}")
        nc.tensor.matmul(ps, lhsT=qT[:, tb * 128 : (tb + 1) * 128], rhs=wv_all,
                         start=True, stop=True)
        nc.vector.tensor_copy(v8[:, tb, :], ps)

    def proj_q(h, qt_h, j):
        ps = psum_wk.tile([128, NC], F32, tag="work", name=f"qp{h}_{j}")
        nc.tensor.matmul(ps, lhsT=wq[:, h, :], rhs=qT[:, j * 512 : (j + 1) * 512],
                         start=True, stop=True)
        nc.vector.tensor_scalar_add(qt_h[:, j * 512 : (j + 1) * 512], ps,
                                    bq[:, h : h + 1])

    def proj_k(h, kt_h, j):
        ps = psum_wk.tile([128, NC], F32, tag="work", name=f"kp{h}_{j}")
        nc.tensor.matmul(ps, lhsT=wk[:, h, :], rhs=qT[:, j * 512 : (j + 1) * 512],
                         start=True, stop=True)
        nc.vector.tensor_copy(kt_h[:, j * 512 : (j + 1) * 512], ps)

    def alloc_head(h):
        qt_h = heads.tile([128, S], F16, tag="QT", name=f"qt{h}")  # [f, s]
        kt_h = heads.tile([128, S], F16, tag="KT", name=f"kt{h}")  # [f, t]
        return qt_h, kt_h

    # prologue: only what the first scores matmul needs, evacuated on the
    # (otherwise idle) ScalarE so the DVE isn't on the critical path
    cur = alloc_head(0)
    ps = psum_wk.tile([128, NC], F32, tag="work", name="qp0_0p")
    nc.tensor.matmul(ps, lhsT=wq[:, 0, :], rhs=qT[:, 0:512], start=True, stop=True)
    nc.scalar.add(cur[0][:, 0:512], ps, bq[:, 0:1])
    ps = psum_wk.tile([128, NC], F32, tag="work", name="kp0_0p")
    nc.tensor.matmul(ps, lhsT=wk[:, 0, :], rhs=qT[:, 0:512], start=True, stop=True)
    nc.vector.tensor_copy(cur[1][:, 0:512], ps)  # DVE, parallel to ScalarE adds
    ps = psum_wk.tile([128, NC], F32, tag="work", name="qp0_1p")
    nc.tensor.matmul(ps, lhsT=wq[:, 0, :], rhs=qT[:, 512:1024], start=True, stop=True)
    nc.scalar.add(cur[0][:, 512:1024], ps, bq[:, 0:1])

    osb_tiles = {}

    def out_proj(h, sh, c, ztn):
        # output projection + head accumulation for chunk c of segment (h, sh)
        wo_ps = psum_wk.tile([128, NC], F32, tag="work", name=f"wop{h}{sh}{c}")
        nc.tensor.matmul(wo_ps, lhsT=wo[:, h, :], rhs=ztn, start=True, stop=True)
        asl = slice(c * 512, (c + 1) * 512)
        if h == 0:
            nc.vector.tensor_copy(acc_a[sh][:, asl], wo_ps)
        elif h == 1:
            nc.vector.tensor_add(acc_b[sh][:, asl], acc_a[sh][:, asl], wo_ps)
        elif h == 2:
            nc.vector.tensor_add(acc_a[sh][:, asl], acc_b[sh][:, asl], wo_ps)
        else:
            if sh not in osb_tiles:
                osb_tiles[sh] = works.tile([128, SW], F16, tag="osb",
                                           name=f"osb{sh}")
            osb = osb_tiles[sh]
            nc.vector.tensor_add(osb[:, asl], acc_a[sh][:, asl], wo_ps)
            if c == 1:
                ssl = slice(sh * SW, (sh + 1) * SW)
                nc.sync.dma_start(out=out_d[:, ssl], in_=osb)

    nxt = None
    deferred = None  # tail work from the previous segment
    for h in range(NH):
        qt_h, kt_h = cur
        for sh in range(2):
            # per-iteration emission schedule. pre[tb] runs at the top of
            # iteration tb (projection drip / deferred out-proj: deps are
            # already met, so they never head-block the FIFO); post[tb] runs
            # after scores+exp of iteration tb (AV/dns pairs whose exp dep
            # clears exactly when the PE FIFO reaches them).
            pre = [[] for _ in range(TB + 1)]
            post = [[] for _ in range(TB + 1)]

            def place(tb, thunk):
                pre[min(tb, TB)].append(thunk)

            def place_post(tb, thunk):
                post[min(tb, TB)].append(thunk)

            # tail of the previous segment: its last matmuls go after this
            # segment's sc(1) (same exp gate — sc(1) must issue first); then
            # per-chunk chains norm0 / wo0 / norm1 / wo1 so chunk 0's work
            # bank recycles as early as possible.
            # previous segment's tail, then its normalization, then its
            # out-projection, then this segment's projection drip, then the
            # denominator accumulation — strictly in that order so the two
            # rotating work banks never force a FIFO head-block.
            base = 0
            if deferred is not None:
                n_mms = len(deferred["mms"])
                for i, t in enumerate(deferred["mms"]):
                    place_post(1 + i // 2, t)
                base = 1 + (n_mms + 1) // 2
                place_post(base, deferred["norm"][0])
                place_post(base + 1, deferred["wo"][0])
                place_post(base + 1, deferred["norm"][1])
                place_post(base + 2, deferred["wo"][1])
                deferred = None

            drip = deque()
            drip0 = 0
            if h == 0 and sh == 0:
                vb = lambda t: (lambda tt=t: v_block(tt))
                pk = lambda j: (lambda jj=j: proj_k(0, kt_h, jj))
                pq = lambda j: (lambda jj=j: proj_q(0, qt_h, jj))
                # V0/V1 borrow the (idle until slot 3) zts banks so they
                # don't wait on the prologue's work-bank evacuations
                vz = lambda t: (lambda tt=t: v_block(tt, psum_av, "zt"))
                drip += [vz(0), vz(1), pk(1), vb(2), vb(3), pk(2), vb(4), vb(5),
                         pq(2), vb(6), vb(7), pk(3), vb(8), vb(9), pq(3)]
                drip += [vb(t) for t in range(10, TB)]
            if sh == 1 and h + 1 < NH:
                nxt = alloc_head(h + 1)
                hh, nq, nk = h + 1, nxt[0], nxt[1]
                drip += [lambda j=j: proj_q(hh, nq, j) for j in range(4)]
                drip += [lambda j=j: proj_k(hh, nk, j) for j in range(4)]
                drip0 = 4
            n_drip = len(drip)
            for i, t in enumerate(drip):
                place_post(drip0 + i // 2, t)
            first_free = max(drip0 + (n_drip + 1) // 2, 5, base + 3)

            s0 = sh * SW
            attnT = attns.tile([128, TB, SW], F8, tag="attnT", name=f"at{h}{sh}")
            zts = [
                psum_av.tile([128, NC], F32, tag="zt", name=f"zt{h}{sh}{c}")
                for c in range(2)
            ]
            dn_tiles = {}

            def dns_pair(p, c, at=attnT, dn=dn_tiles, hh=h, ss=sh):
                if c not in dn:
                    dn[c] = psum_wk.tile(
                        [128, NC], F32, tag="work", name=f"dn{hh}{ss}{c}"
                    )
                nc.tensor.matmul(
                    dn[c],
                    lhsT=ones8,
                    rhs=at[:, 2 * p : 2 * p + 2, c * 512 : (c + 1) * 512],
                    start=(p == 0), stop=(p == 7), perf_mode=DR,
                )

            def av_pair(p, c, at=attnT, z=zts, hh=h):
                nc.tensor.matmul(
                    z[c],
                    lhsT=v8[:, 2 * p : 2 * p + 2, hh * 128 : (hh + 1) * 128],
                    rhs=at[:, 2 * p : 2 * p + 2, c * 512 : (c + 1) * 512],
                    start=(p == 0), stop=(p == 7), perf_mode=DR,
                )

            for p in range(8):
                place_post(
                    max(2 * p + 3, first_free + p),
                    lambda p=p, f=dns_pair: (f(p, 0), f(p, 1)),
                )
                place_post(
                    2 * p + 3, lambda p=p, f=av_pair: (f(p, 0), f(p, 1))
                )

            for tb in range(TB):
                for t in pre[tb]:
                    t()
                sc = psum_sc.tile([128, SW], F32, tag="sc", name=f"sc{h}{sh}{tb}")
                for c in range(2):
                    nc.tensor.matmul(
                        sc[:, c * 512 : (c + 1) * 512],
                        lhsT=kt_h[:, tb * 128 : (tb + 1) * 128],
                        rhs=qt_h[:, s0 + c * 512 : s0 + (c + 1) * 512],
                        start=True, stop=True,
                    )
                nc.scalar.activation(attnT[:, tb, :], sc, EXP, scale=SCALE)
                for t in post[tb]:
                    t()

            holder = {}

            def norm_chunk(c, dn=dn_tiles, z=zts, hol=holder, hh=h, ss=sh):
                recip = works.tile([128, NC], F32, tag="recip",
                                   name=f"rc{hh}{ss}{c}")
                nc.vector.reciprocal_approx_fast(recip, dn[c])
                ztn = works.tile([128, NC], F16, tag="ztn", name=f"zn{hh}{ss}{c}")
                nc.vector.tensor_mul(ztn, z[c], recip)
                hol[c] = ztn

            if h == NH - 1 and sh == 1:
                # final segment: drain immediately, complete per-chunk chains
                for t in pre[TB]:
                    t()
                for t in post[TB]:
                    t()
                for c in range(2):
                    norm_chunk(c)
                    out_proj(h, sh, c, holder[c])
            else:
                deferred = {
                    "mms": pre[TB] + post[TB],
                    "norm": [lambda c=c, f=norm_chunk: f(c) for c in range(2)],
                    "wo": [
                        lambda c=c, hol=holder, hh=h, ss=sh: out_proj(
                            hh, ss, c, hol[c]
                        )
                        for c in range(2)
                    ],
                }
        if h + 1 < NH:
            cur = nxt

    ctx.close()


def _in_maps(inputs):
    q = np.asarray(inputs["q"], dtype=np.float32)
    Wq = np.asarray(inputs["Wq"], dtype=np.float32)
    bq = np.asarray(inputs["bq"], dtype=np.float32)
    Wk = np.asarray(inputs["Wk"], dtype=np.float32)
    Wv = np.asarray(inputs["Wv"], dtype=np.float32)
    Wo = np.asarray(inputs["Wo"], dtype=np.float32).reshape(H, E, E)

    def warr(w, hs):  # [h, e_in, e_out] slice -> [e_in, h, e_out] f16
        return w[hs].transpose(1, 0, 2).astype(np.float16)

    maps = []
    for c in range(8):
        b = c // 2
        hs = slice(4 * (c % 2), 4 * (c % 2) + 4)
        w_all = np.ascontiguousarray(
            np.stack([warr(Wq, hs), warr(Wk, hs), warr(Wv, hs), warr(Wo, hs)], 1)
        )  # [e_in, 4, h, e_out]
        maps.append(
            {
                "qT": np.ascontiguousarray(q[b].T).astype(np.float16),
                "W": w_all,
                "bq": np.ascontiguousarray(bq[hs].T),
            }
        )
    return maps


def kernel(**inputs):
    nc = build_program()
    maps = _in_maps(inputs)
    res = run_bass_kernel_spmd(nc, maps, core_ids=list(range(8)))
    bo = np.asarray(inputs["bo"], dtype=np.float32)
    bv = np.asarray(inputs["bv"], dtype=np.float32)
    Wo = np.asarray(inputs["Wo"], dtype=np.float32).reshape(H, E, E)
    # V-bias contribution folded out of the device kernel:
    # sum_h softmax(..)@ (qWv + bv) @ Wo_h = device_partials + sum_h bv_h @ Wo_h
    bo_eff = bo + np.einsum("he,hef->f", bv, Wo).astype(np.float32)
    out = np.empty((B, S, E), dtype=np.float32)
    for b in range(B):
        part = res.results[2 * b]["out"].astype(np.float32) + res.results[
            2 * b + 1
        ]["out"].astype(np.float32)
        out[b] = part.T + bo_eff
    return out
